# revision 25
# baseline (speedup 1.0000x reference)
"""Trainium2 Bass kernel v2 for a bilinear field-interaction layer.

Computation (per example b):
  v[f]   = fields[f, b] @ W + bw                       # shared Dense(D)
  p[i,j] = dot(v[i], fields[j, b])  for i < j          # 780 pairs
  out[b] = p @ Wc + bc                                 # Dense(OUT)

Shapes: fields [40, 16384, 32], W [32, 32], Wc [780, 50] -> out [16384, 50].

v3 strategy (8 NeuronCores, batch-sharded 2048/core, bf16 compute):
  - Host: cast fields to bf16 and pre-tile as [nt*1280, 128] slabs so one
    dma_start_transpose (DMA xbar, ~90% of DMA BW) per b-tile yields
    x_t[32q+d, 40w+f] directly -- no on-chip StreamTranspose.  The (w, f)
    column order makes each example's 40 field-columns CONTIGUOUS, which
    is critical: strided matmul rhs streams ~4-5x slower than contiguous.
  - step1: one K=128 matmul per 512-col group with a block-diagonal
    W replicated 4x (w_bd), full-array FWL-eligible bf16 weights.
  - pairwise: per-example G[j,i] = x_j . v_i matmuls, slot-major loop so
    consecutive instructions land on 8 disjoint PE tiles (4 row-groups x
    2 col-groups) and execute concurrently; lhsT/rhs contiguous 40-col.
  - s_sb layout i-OUTERMOST: s_sb[(c2,j), i*EXc + ex] so each projection
    round's rhs is one contiguous EXc-column block (gens merged).
  - projection: 40 i-rounds accumulating Wc_i^T @ S_i into PSUM, the two
    c2 halves on diagonal quadrants running concurrently, N=EXc contig.
  - out: PE-transpose back to [b, o], +bc on evacuation, batched DMA out.
"""

import sys

sys.path.insert(0, "/opt/trn_rl_repo")

from contextlib import ExitStack

import ml_dtypes
import numpy as np

import concourse.bass as bass
import concourse.tile as tile
from concourse import mybir
from concourse._compat import with_exitstack
from concourse.bass_utils import run_bass_kernel_spmd

F, D, OUT = 40, 32, 50
FD = F * D  # 1280
NPAIR = F * (F - 1) // 2
N_CORES = 8
B_FULL = 16384
BC = B_FULL // N_CORES  # 2048 per core
CHUNK_T = 4             # b-tiles per final-stage chunk
GENS = ((0, 12), (1, 4))  # (gen index, slots per c2); 2*(12+4)=32 = b-block size
SPT = 2560              # s_sb free size per b-tile: genA 4q*480 + genB 4q*160
HOST_V = False          # if True, v is computed on host and DMA'd in
LDW_OPT = False         # if True, compile with walrus --enable-ldw-opt=true

BF16 = ml_dtypes.bfloat16


def _apply_ldw_opt_patch():
    """Rewrite the hardcoded --enable-ldw-opt=false walrus flag so LDWEIGHTS
    can be hoisted to the background weight buffer (pipelines with in-flight
    matmuls on the same PE tile)."""
    import concourse.bass_utils as _bu
    if getattr(_bu, "_ldw_patched", False):
        return
    orig = _bu.run_command

    def patched(cmd, *a, **kw):
        if isinstance(cmd, list):
            cmd = ["--enable-ldw-opt=true" if c == "--enable-ldw-opt=false"
                   else c for c in cmd]
        return orig(cmd, *a, **kw)

    _bu.run_command = patched
    _bu._ldw_patched = True


def _gen_geometry(gen):
    nslot = GENS[gen][1]
    sbase = 0 if gen == 0 else 1920      # s_sb free base within a t-slice
    gq = 40 * nslot                      # s_sb q-stride (480 / 160)
    w0 = 0 if gen == 0 else 24           # first w (example-in-block) of the gen
    return nslot, sbase, gq, w0


@with_exitstack
def build_kernel(ctx: ExitStack, tc: tile.TileContext, out_ext, xh_ext,
                 wct_ext, ident_ext, bcr_ext, wbd_ext=None, bwr_ext=None,
                 vh_ext=None, bc_count=BC, bw_is_zero=True):
    nc = tc.nc
    f32 = mybir.dt.float32
    bf16 = mybir.dt.bfloat16
    nt = bc_count // 128
    n_chunks = (nt + CHUNK_T - 1) // CHUNK_T
    host_v = vh_ext is not None

    const = ctx.enter_context(tc.tile_pool(name="const", bufs=1))
    xpool = ctx.enter_context(tc.tile_pool(name="xpool", bufs=3))
    vpool = ctx.enter_context(tc.tile_pool(name="vpool", bufs=3))
    spool = ctx.enter_context(tc.tile_pool(name="spool", bufs=2))
    apool = ctx.enter_context(tc.tile_pool(name="apool", bufs=2))
    opool = ctx.enter_context(tc.tile_pool(name="opool", bufs=2))
    gps = ctx.enter_context(tc.tile_pool(name="gps", bufs=1, space="PSUM"))
    vps = ctx.enter_context(tc.tile_pool(name="vps", bufs=2, space="PSUM"))
    accps = ctx.enter_context(tc.tile_pool(name="accps", bufs=1, space="PSUM"))
    ops = ctx.enter_context(tc.tile_pool(name="ops", bufs=1, space="PSUM"))

    # ---- constants
    wct = const.tile([128, F * OUT], bf16)
    nc.sync.dma_start(wct[:], wct_ext[:])
    ident = const.tile([128, OUT], f32)
    nc.sync.dma_start(ident[:], ident_ext[:])
    bc_rep = const.tile([128, OUT], f32)
    nc.sync.dma_start(bc_rep[:], bcr_ext[:])
    if not host_v:
        w_bd = const.tile([128, 128], bf16)
        nc.sync.dma_start(w_bd[:], wbd_ext[:])
        bw_rep = const.tile([128, 1], f32)
        nc.sync.dma_start(bw_rep[:], bwr_ext[:])

    # out viewed [t, q, w, o]
    out_r3 = out_ext.rearrange("(t q w) o -> t q w o", q=4, w=32)

    # ---- persistent pairwise PSUM banks (gap rows inited once)
    g_ps = [gps.tile([128, 480], f32, name=f"gps{q}") for q in range(4)]
    for q in range(4):
        nc.vector.memset(g_ps[q][32:64, :], 0.0)

    # evac engine assignment: big copies to ACT (352-cyc overhead amortized),
    # small ones to DVE (58-cyc overhead); keep both balanced.
    def copy_act(dst, src):
        nc.scalar.copy(dst, src)

    def copy_dve(dst, src):
        nc.vector.tensor_copy(dst, src)

    def proj_and_out(s_sb, t0, tn, exc):
        # ---- projection: acc[64c2+o, c2*exc + ex] += wct_i^T @ S_i
        acc = accps.tile([128, 2 * exc], f32, name="acc", tag="acc")
        for i in range(F):
            for c2 in range(2):
                p0 = 64 * c2
                nc.tensor.matmul(acc[p0:p0+OUT, c2*exc:(c2+1)*exc],
                                 wct[p0:p0+F, i*OUT:(i+1)*OUT],
                                 s_sb[p0:p0+F, i*exc:(i+1)*exc],
                                 start=(i == 0), stop=(i == F - 1),
                                 tile_position=(p0, p0),
                                 skip_group_check=True)

        # ---- merge parities, PE-transpose to [b, o], +bc, scatter out
        for c2 in range(2):
            p0 = 64 * c2
            a_sb = apool.tile([128, exc], f32, name=f"a_sb{c2}",
                              tag=f"a_sb{c2}")
            copy_dve(a_sb[p0:p0+OUT, :], acc[p0:p0+OUT, c2*exc:(c2+1)*exc])
            for k in range((exc + 127) // 128):
                w = min(128, exc - 128 * k)
                o_ps = ops.tile([128, OUT], f32, name="o_ps", tag="o_ps")
                nc.tensor.matmul(o_ps[0:w, :], a_sb[p0:p0+OUT, 128*k:128*k+w],
                                 ident[p0:p0+OUT, :], is_transpose=True,
                                 start=True, stop=True,
                                 tile_position=(p0, 0),
                                 skip_group_check=True)
                o_sb = opool.tile([128, OUT], f32, name="o_sb", tag="o_sb")
                nc.vector.tensor_add(o_sb[0:w, :], o_ps[0:w, :],
                                     bc_rep[0:w, :])
                # rows: (t'loc, gen-block, q, slot); DMA per (t'loc, gen)
                for tl in range(w // 64):
                    t_ = t0 + 2 * k + tl
                    for gen, nslot in GENS:
                        genb = 0 if gen == 0 else 48
                        wb = (12 * c2) if gen == 0 else (24 + 4 * c2)
                        src = o_sb[tl*64+genb : tl*64+genb+4*nslot, :]
                        dst = out_r3[t_, :, wb:wb+nslot, :]
                        nc.sync.dma_start(dst, src)

    prev = None
    for chunk in range(n_chunks):
        t0 = chunk * CHUNK_T
        tn = min(CHUNK_T, nt - t0)
        exc = tn * 64  # example-columns per c2 half of the chunk
        # s_sb[(c2,j), i*exc + ex], ex = t'*64 + genbase + q*nslot + slot
        s_sb = spool.tile([128, F * exc], bf16, name="s_sb", tag="s_sb")

        for tt in range(tn):
            t = t0 + tt
            # ---- load with DMA-xbar transpose: x_t[32q+d, 40w+f]
            x_t = xpool.tile([128, FD], bf16, name="x_t", tag="x_t")
            xq = nc.sync if (t % 2 == 0) else nc.scalar
            xq.dma_start_transpose(x_t[:], xh_ext[t * FD:(t + 1) * FD, :])

            if host_v:
                v_t = vpool.tile([128, FD], bf16, name="v_t", tag="v_t")
                nc.scalar.dma_start_transpose(
                    v_t[:], vh_ext[t * FD:(t + 1) * FD, :])
            else:
                # ---- step1: v^T = blockdiag(W)^T @ x^T (+bw), K=128 M=128
                v_t = vpool.tile([128, FD], bf16, name="v_t", tag="v_t")
                for s in range(3):
                    n0, n1 = s * 512, min(FD, (s + 1) * 512)
                    w = n1 - n0
                    v_ps = vps.tile([128, 512], f32, name="v_ps", tag="v_ps")
                    nc.tensor.matmul(v_ps[:, :w], w_bd[:], x_t[:, n0:n1],
                                     start=True, stop=True)
                    if bw_is_zero:
                        if s < 2:
                            copy_act(v_t[:, n0:n1], v_ps[:, :w])
                        else:
                            copy_dve(v_t[:, n0:n1], v_ps[:, :w])
                    else:
                        nc.vector.tensor_scalar_add(v_t[:, n0:n1],
                                                    v_ps[:, :w],
                                                    bw_rep[:, 0:1])

            # ---- pairwise: G[j, i] = x_j . v_i  per example
            # slot-major order: consecutive MMs hit 8 disjoint PE tiles.
            s_r = s_sb[0:104, :].rearrange("p (e x) -> p e x", e=F)
            for gen, nslot in GENS:
                _, _, gq, w0 = _gen_geometry(gen)
                for slot in range(nslot):
                    for q in range(4):
                        for c2 in range(2):
                            bp = w0 + c2 * nslot + slot
                            sl = slice(40 * bp, 40 * bp + 40)
                            nc.tensor.matmul(
                                g_ps[q][64*c2:64*c2+F, 40*slot:40*slot+F],
                                x_t[32*q:32*q+32, sl],
                                v_t[32*q:32*q+32, sl],
                                start=True, stop=True,
                                tile_position=(32*q, 64*c2))
                genb = 0 if gen == 0 else 48
                # evac: one copy per q bank into i-outer s_sb layout
                for q in range(4):
                    src = g_ps[q][0:104, 0:gq]
                    src = src.rearrange("p (s e) -> p e s", s=nslot)
                    base = tt * 64 + genb + q * nslot
                    dst = s_r[:, :, base:base+nslot]
                    if gen == 0 and q < 2:
                        copy_act(dst, src)
                    else:
                        copy_dve(dst, src)

        # software pipeline: emit the previous chunk's projection AFTER this
        # chunk's tile work so the in-order PE queue never stalls waiting for
        # the current chunk's evacuations.
        if prev is not None:
            proj_and_out(*prev)
        prev = (s_sb, t0, tn, exc)
    proj_and_out(*prev)


def _host_prep_shared(W, bw, Wc, bc):
    W = np.asarray(W, np.float32)
    w_bd = np.zeros((128, 128), np.float32)
    for q in range(4):
        w_bd[32*q:32*q+32, 32*q:32*q+32] = W
    w_bd = w_bd.astype(BF16)

    iu, ju = np.triu_indices(F, k=1)
    WcFull = np.zeros((F, F, OUT), np.float32)
    WcFull[iu, ju] = np.asarray(Wc, np.float32)
    wT = np.ascontiguousarray(np.transpose(WcFull, (1, 0, 2))).reshape(F, F*OUT)
    wct = np.zeros((128, F * OUT), np.float32)
    wct[0:40] = wT
    wct[64:104] = wT
    wct = wct.astype(BF16)

    ident = np.zeros((128, OUT), np.float32)
    for p in range(OUT):
        ident[p, p] = 1.0
        ident[64 + p, p] = 1.0
    bc_rep = np.tile(np.asarray(bc, np.float32).reshape(1, -1), (128, 1))
    bw_rep = np.tile(np.asarray(bw, np.float32).reshape(-1, 1), (4, 1))
    return w_bd, wct, ident, bc_rep, bw_rep


def _host_tile_slab(arr_f_b_d, bc_count):
    """[F, bc, D] fp32/bf16 -> [nt*1280, 128] bf16 slab for dma_start_transpose.

    Row order (w, f) within a tile so the transposed SBUF image is
    x_t[32q+d, 40w+f]: each example's 40 field-columns contiguous."""
    nt = bc_count // 128
    a = np.asarray(arr_f_b_d)
    a = a.reshape(F, nt, 4, 32, D).transpose(1, 3, 0, 2, 4)
    return np.ascontiguousarray(a.reshape(nt * FD, 128).astype(BF16))


_WAIT_CAPS = {}
_WAIT_CAP_DEFAULT = 1


def legalize_waits(nc):
    """Walrus codegen accepts only a limited number of sync-wait commands per
    instruction (1 for matmul's S3_LW path, 2 for most others).  Hoist excess
    waits onto no-op instructions on the same engine immediately before."""
    for fn in nc.m.functions:
        for blk in fn.blocks:
            out = []
            for inst in blk.instructions:
                si = getattr(inst, "sync_info", None)
                waits = list(si.on_wait) if si is not None and si.on_wait else []
                cap = _WAIT_CAPS.get(type(inst).__name__, _WAIT_CAP_DEFAULT)
                if len(waits) > cap:
                    excess, keep = waits[:-cap], waits[-cap:]
                    for ci, w in enumerate(excess):
                        nop = mybir.InstNoOp(
                            name=f"{inst.name}-waitsplit{ci}",
                            sync_info=mybir.SyncInfo(on_wait=[w], on_update=[]),
                            bass_nofuse=True,
                            engine=inst.engine,
                        )
                        out.append(nop)
                    si.on_wait = keep
                out.append(inst)
            blk.instructions[:] = out


def make_nc(bc_count=BC, bw_is_zero=True, host_v=HOST_V, legalize=True):
    nc = bass.Bass()
    nt = bc_count // 128
    bf = mybir.dt.bfloat16
    f32 = mybir.dt.float32
    xh_ext = nc.declare_dram_parameter("xh", [nt * FD, 128], bf, isOutput=False)
    wct_ext = nc.declare_dram_parameter("wct", [128, F * OUT], bf,
                                        isOutput=False)
    ident_ext = nc.declare_dram_parameter("ident", [128, OUT], f32,
                                          isOutput=False)
    bcr_ext = nc.declare_dram_parameter("bc_rep", [128, OUT], f32,
                                        isOutput=False)
    kw = {}
    if host_v:
        kw["vh_ext"] = nc.declare_dram_parameter("vh", [nt * FD, 128], bf,
                                                 isOutput=False)
    else:
        kw["wbd_ext"] = nc.declare_dram_parameter("w_bd", [128, 128], bf,
                                                  isOutput=False)
        kw["bwr_ext"] = nc.declare_dram_parameter("bw_rep", [128, 1], f32,
                                                  isOutput=False)
    out_ext = nc.declare_dram_parameter("out", [bc_count, OUT], f32,
                                        isOutput=True)
    with tile.TileContext(nc) as tc:
        build_kernel(tc, out_ext, xh_ext, wct_ext, ident_ext, bcr_ext,
                     bc_count=bc_count, bw_is_zero=bw_is_zero, **kw)
    if legalize:
        legalize_waits(nc)
    return nc


def make_in_maps(fields, W, bw, Wc, bc, bc_count=BC, n_cores=N_CORES,
                 host_v=HOST_V):
    fields = np.asarray(fields, np.float32)
    w_bd, wct, ident, bc_rep, bw_rep = _host_prep_shared(W, bw, Wc, bc)
    if host_v:
        v_full = (np.einsum("fbd,de->fbe", fields.astype(BF16).astype(np.float32),
                            np.asarray(W, np.float32).astype(BF16).astype(np.float32))
                  + np.asarray(bw, np.float32)).astype(np.float32)
    in_maps = []
    for c in range(n_cores):
        sl = fields[:, c*bc_count:(c+1)*bc_count, :]
        m = {
            "xh": _host_tile_slab(sl, bc_count),
            "wct": wct, "ident": ident, "bc_rep": bc_rep,
        }
        if host_v:
            m["vh"] = _host_tile_slab(v_full[:, c*bc_count:(c+1)*bc_count, :],
                                      bc_count)
        else:
            m["w_bd"] = w_bd
            m["bw_rep"] = bw_rep
        in_maps.append(m)
    return in_maps


def run_cores(fields, W, bw, Wc, bc, bc_count=BC, n_cores=N_CORES,
              host_v=HOST_V, trace=False, tmpdir=None, ldw_opt=None):
    if ldw_opt if ldw_opt is not None else LDW_OPT:
        _apply_ldw_opt_patch()
    bw_is_zero = bool(np.all(np.asarray(bw) == 0))
    nc = make_nc(bc_count, bw_is_zero=bw_is_zero, host_v=host_v)
    in_maps = make_in_maps(fields, W, bw, Wc, bc, bc_count, n_cores, host_v)
    res = run_bass_kernel_spmd(nc, in_maps, list(range(n_cores)), trace=trace,
                               tmpdir=tmpdir)
    outs = [res.results[c]["out"] for c in range(n_cores)]
    return np.concatenate(outs, axis=0).astype(np.float32), res


def kernel(fields, W, bw, Wc, bc):
    out, _ = run_cores(fields, W, bw, Wc, bc)
    return out


# revision 26
# speedup vs baseline: 1.1896x; 1.1896x over previous
"""Trainium2 Bass kernel v2 for a bilinear field-interaction layer.

Computation (per example b):
  v[f]   = fields[f, b] @ W + bw                       # shared Dense(D)
  p[i,j] = dot(v[i], fields[j, b])  for i < j          # 780 pairs
  out[b] = p @ Wc + bc                                 # Dense(OUT)

Shapes: fields [40, 16384, 32], W [32, 32], Wc [780, 50] -> out [16384, 50].

v3 strategy (8 NeuronCores, batch-sharded 2048/core, bf16 compute):
  - Host: cast fields to bf16 and pre-tile as [nt*1280, 128] slabs so one
    dma_start_transpose (DMA xbar, ~90% of DMA BW) per b-tile yields
    x_t[32q+d, 40w+f] directly -- no on-chip StreamTranspose.  The (w, f)
    column order makes each example's 40 field-columns CONTIGUOUS, which
    is critical: strided matmul rhs streams ~4-5x slower than contiguous.
  - step1: one K=128 matmul per 512-col group with a block-diagonal
    W replicated 4x (w_bd), full-array FWL-eligible bf16 weights.
  - pairwise: per-example G[j,i] = x_j . v_i matmuls, slot-major loop so
    consecutive instructions land on 8 disjoint PE tiles (4 row-groups x
    2 col-groups) and execute concurrently; lhsT/rhs contiguous 40-col.
  - s_sb layout i-OUTERMOST: s_sb[(c2,j), i*EXc + ex] so each projection
    round's rhs is one contiguous EXc-column block (gens merged).
  - projection: 40 i-rounds accumulating Wc_i^T @ S_i into PSUM, the two
    c2 halves on diagonal quadrants running concurrently, N=EXc contig.
  - out: PE-transpose back to [b, o], +bc on evacuation, batched DMA out.
"""

import sys

sys.path.insert(0, "/opt/trn_rl_repo")

from contextlib import ExitStack

import ml_dtypes
import numpy as np

import concourse.bass as bass
import concourse.tile as tile
from concourse import mybir
from concourse._compat import with_exitstack
from concourse.bass_utils import run_bass_kernel_spmd

F, D, OUT = 40, 32, 50
FD = F * D  # 1280
NPAIR = F * (F - 1) // 2
N_CORES = 8
B_FULL = 16384
BC = B_FULL // N_CORES  # 2048 per core
CHUNK_T = 4             # b-tiles per final-stage chunk
GENS = ((0, 12), (1, 4))  # (gen index, slots per c2); 2*(12+4)=32 = b-block size
SPT = 2560              # s_sb free size per b-tile: genA 4q*480 + genB 4q*160
HOST_V = False          # if True, v is computed on host and DMA'd in
LDW_OPT = False         # if True, compile with walrus --enable-ldw-opt=true

BF16 = ml_dtypes.bfloat16


def _apply_ldw_opt_patch():
    """Rewrite the hardcoded --enable-ldw-opt=false walrus flag so LDWEIGHTS
    can be hoisted to the background weight buffer (pipelines with in-flight
    matmuls on the same PE tile)."""
    import concourse.bass_utils as _bu
    if getattr(_bu, "_ldw_patched", False):
        return
    orig = _bu.run_command

    def patched(cmd, *a, **kw):
        if isinstance(cmd, list):
            cmd = ["--enable-ldw-opt=true" if c == "--enable-ldw-opt=false"
                   else c for c in cmd]
        return orig(cmd, *a, **kw)

    _bu.run_command = patched
    _bu._ldw_patched = True


def _gen_geometry(gen):
    nslot = GENS[gen][1]
    sbase = 0 if gen == 0 else 1920      # s_sb free base within a t-slice
    gq = 40 * nslot                      # s_sb q-stride (480 / 160)
    w0 = 0 if gen == 0 else 24           # first w (example-in-block) of the gen
    return nslot, sbase, gq, w0


@with_exitstack
def build_kernel(ctx: ExitStack, tc: tile.TileContext, out_ext, xh_ext,
                 wct_ext, ident_ext, bcr_ext, wbd_ext=None, bwr_ext=None,
                 vh_ext=None, bc_count=BC, bw_is_zero=True):
    nc = tc.nc
    f32 = mybir.dt.float32
    bf16 = mybir.dt.bfloat16
    nt = bc_count // 128
    n_chunks = (nt + CHUNK_T - 1) // CHUNK_T
    host_v = vh_ext is not None

    const = ctx.enter_context(tc.tile_pool(name="const", bufs=1))
    xpool = ctx.enter_context(tc.tile_pool(name="xpool", bufs=3))
    vpool = ctx.enter_context(tc.tile_pool(name="vpool", bufs=3))
    spool = ctx.enter_context(tc.tile_pool(name="spool", bufs=2))
    apool = ctx.enter_context(tc.tile_pool(name="apool", bufs=2))
    opool = ctx.enter_context(tc.tile_pool(name="opool", bufs=2))
    gps = ctx.enter_context(tc.tile_pool(name="gps", bufs=1, space="PSUM"))
    vps = ctx.enter_context(tc.tile_pool(name="vps", bufs=2, space="PSUM"))
    accps = ctx.enter_context(tc.tile_pool(name="accps", bufs=1, space="PSUM"))
    ops = ctx.enter_context(tc.tile_pool(name="ops", bufs=1, space="PSUM"))

    # ---- constants
    wct = const.tile([128, F * OUT], bf16)
    nc.sync.dma_start(wct[:], wct_ext[:])
    ident = const.tile([128, OUT], f32)
    nc.sync.dma_start(ident[:], ident_ext[:])
    bc_rep = const.tile([128, OUT], f32)
    nc.sync.dma_start(bc_rep[:], bcr_ext[:])
    if not host_v:
        w_bd = const.tile([128, 128], bf16)
        nc.sync.dma_start(w_bd[:], wbd_ext[:])
        bw_rep = const.tile([128, 1], f32)
        nc.sync.dma_start(bw_rep[:], bwr_ext[:])

    # out viewed [t, q, w, o]
    out_r3 = out_ext.rearrange("(t q w) o -> t q w o", q=4, w=32)

    # ---- persistent pairwise PSUM banks (gap rows inited once)
    g_ps = [gps.tile([128, 480], f32, name=f"gps{q}") for q in range(4)]
    for q in range(4):
        nc.vector.memset(g_ps[q][32:64, :], 0.0)

    # evac engine assignment: big copies to ACT (352-cyc overhead amortized),
    # small ones to DVE (58-cyc overhead); keep both balanced.
    def copy_act(dst, src):
        nc.scalar.copy(dst, src)

    def copy_dve(dst, src):
        nc.vector.tensor_copy(dst, src)

    def proj_and_out(s_sb, t0, tn, exc):
        # ---- projection: acc[64c2+o, c2*exc + ex] += wct_i^T @ S_i
        acc = accps.tile([128, 2 * exc], f32, name="acc", tag="acc")
        for i in range(F):
            for c2 in range(2):
                p0 = 64 * c2
                nc.tensor.matmul(acc[p0:p0+OUT, c2*exc:(c2+1)*exc],
                                 wct[p0:p0+F, i*OUT:(i+1)*OUT],
                                 s_sb[p0:p0+F, i*exc:(i+1)*exc],
                                 start=(i == 0), stop=(i == F - 1),
                                 tile_position=(p0, p0),
                                 skip_group_check=True)

        # ---- merge parities, PE-transpose to [b, o], +bc, scatter out
        for c2 in range(2):
            p0 = 64 * c2
            a_sb = apool.tile([128, exc], f32, name=f"a_sb{c2}",
                              tag=f"a_sb{c2}")
            copy_dve(a_sb[p0:p0+OUT, :], acc[p0:p0+OUT, c2*exc:(c2+1)*exc])
            for k in range((exc + 127) // 128):
                w = min(128, exc - 128 * k)
                o_ps = ops.tile([128, OUT], f32, name="o_ps", tag="o_ps")
                nc.tensor.matmul(o_ps[0:w, :], a_sb[p0:p0+OUT, 128*k:128*k+w],
                                 ident[p0:p0+OUT, :], is_transpose=True,
                                 start=True, stop=True,
                                 tile_position=(p0, 0),
                                 skip_group_check=True)
                o_sb = opool.tile([128, OUT], f32, name="o_sb", tag="o_sb")
                nc.vector.tensor_add(o_sb[0:w, :], o_ps[0:w, :],
                                     bc_rep[0:w, :])
                # rows: (t'loc, gen-block, q, slot); DMA per (t'loc, gen)
                for tl in range(w // 64):
                    t_ = t0 + 2 * k + tl
                    for gen, nslot in GENS:
                        genb = 0 if gen == 0 else 48
                        wb = (12 * c2) if gen == 0 else (24 + 4 * c2)
                        src = o_sb[tl*64+genb : tl*64+genb+4*nslot, :]
                        dst = out_r3[t_, :, wb:wb+nslot, :]
                        nc.sync.dma_start(dst, src)

    prev = None
    for chunk in range(n_chunks):
        t0 = chunk * CHUNK_T
        tn = min(CHUNK_T, nt - t0)
        exc = tn * 64  # example-columns per c2 half of the chunk
        # s_sb[(c2,j), i*exc + ex], ex = t'*64 + genbase + q*nslot + slot
        s_sb = spool.tile([128, F * exc], bf16, name="s_sb", tag="s_sb")

        for tt in range(tn):
            t = t0 + tt
            # ---- load with DMA-xbar transpose: x_t[32q+d, 40w+f]
            x_t = xpool.tile([128, FD], bf16, name="x_t", tag="x_t")
            nc.sync.dma_start_transpose(x_t[:], xh_ext[t * FD:(t + 1) * FD, :])

            if host_v:
                v_t = vpool.tile([128, FD], bf16, name="v_t", tag="v_t")
                nc.scalar.dma_start_transpose(
                    v_t[:], vh_ext[t * FD:(t + 1) * FD, :])
            else:
                # ---- step1: v^T = blockdiag(W)^T @ x^T (+bw), K=128 M=128
                v_t = vpool.tile([128, FD], bf16, name="v_t", tag="v_t")
                for s in range(3):
                    n0, n1 = s * 512, min(FD, (s + 1) * 512)
                    w = n1 - n0
                    v_ps = vps.tile([128, 512], f32, name="v_ps", tag="v_ps")
                    nc.tensor.matmul(v_ps[:, :w], w_bd[:], x_t[:, n0:n1],
                                     start=True, stop=True)
                    if bw_is_zero:
                        if s < 2:
                            copy_act(v_t[:, n0:n1], v_ps[:, :w])
                        else:
                            copy_dve(v_t[:, n0:n1], v_ps[:, :w])
                    else:
                        nc.vector.tensor_scalar_add(v_t[:, n0:n1],
                                                    v_ps[:, :w],
                                                    bw_rep[:, 0:1])

            # ---- pairwise: G[j, i] = x_j . v_i  per example
            # slot-major order: consecutive MMs hit 8 disjoint PE tiles.
            s_r = s_sb[0:104, :].rearrange("p (e x) -> p e x", e=F)
            for gen, nslot in GENS:
                _, _, gq, w0 = _gen_geometry(gen)
                for slot in range(nslot):
                    for q in range(4):
                        for c2 in range(2):
                            bp = w0 + c2 * nslot + slot
                            sl = slice(40 * bp, 40 * bp + 40)
                            nc.tensor.matmul(
                                g_ps[q][64*c2:64*c2+F, 40*slot:40*slot+F],
                                x_t[32*q:32*q+32, sl],
                                v_t[32*q:32*q+32, sl],
                                start=True, stop=True,
                                tile_position=(32*q, 64*c2))
                genb = 0 if gen == 0 else 48
                # evac: one copy per q bank into i-outer s_sb layout
                for q in range(4):
                    src = g_ps[q][0:104, 0:gq]
                    src = src.rearrange("p (s e) -> p e s", s=nslot)
                    base = tt * 64 + genb + q * nslot
                    dst = s_r[:, :, base:base+nslot]
                    if gen == 0 and q < 2:
                        copy_act(dst, src)
                    else:
                        copy_dve(dst, src)

        # software pipeline: emit the previous chunk's projection AFTER this
        # chunk's tile work so the in-order PE queue never stalls waiting for
        # the current chunk's evacuations.
        if prev is not None:
            proj_and_out(*prev)
        prev = (s_sb, t0, tn, exc)
    proj_and_out(*prev)


def _host_prep_shared(W, bw, Wc, bc):
    W = np.asarray(W, np.float32)
    w_bd = np.zeros((128, 128), np.float32)
    for q in range(4):
        w_bd[32*q:32*q+32, 32*q:32*q+32] = W
    w_bd = w_bd.astype(BF16)

    iu, ju = np.triu_indices(F, k=1)
    WcFull = np.zeros((F, F, OUT), np.float32)
    WcFull[iu, ju] = np.asarray(Wc, np.float32)
    wT = np.ascontiguousarray(np.transpose(WcFull, (1, 0, 2))).reshape(F, F*OUT)
    wct = np.zeros((128, F * OUT), np.float32)
    wct[0:40] = wT
    wct[64:104] = wT
    wct = wct.astype(BF16)

    ident = np.zeros((128, OUT), np.float32)
    for p in range(OUT):
        ident[p, p] = 1.0
        ident[64 + p, p] = 1.0
    bc_rep = np.tile(np.asarray(bc, np.float32).reshape(1, -1), (128, 1))
    bw_rep = np.tile(np.asarray(bw, np.float32).reshape(-1, 1), (4, 1))
    return w_bd, wct, ident, bc_rep, bw_rep


def _host_tile_slab(arr_f_b_d, bc_count):
    """[F, bc, D] fp32/bf16 -> [nt*1280, 128] bf16 slab for dma_start_transpose.

    Row order (w, f) within a tile so the transposed SBUF image is
    x_t[32q+d, 40w+f]: each example's 40 field-columns contiguous."""
    nt = bc_count // 128
    a = np.asarray(arr_f_b_d)
    a = a.reshape(F, nt, 4, 32, D).transpose(1, 3, 0, 2, 4)
    return np.ascontiguousarray(a.reshape(nt * FD, 128).astype(BF16))


_WAIT_CAPS = {}
_WAIT_CAP_DEFAULT = 1


def legalize_waits(nc):
    """Walrus codegen accepts only a limited number of sync-wait commands per
    instruction (1 for matmul's S3_LW path, 2 for most others).  Hoist excess
    waits onto no-op instructions on the same engine immediately before."""
    for fn in nc.m.functions:
        for blk in fn.blocks:
            out = []
            for inst in blk.instructions:
                si = getattr(inst, "sync_info", None)
                waits = list(si.on_wait) if si is not None and si.on_wait else []
                cap = _WAIT_CAPS.get(type(inst).__name__, _WAIT_CAP_DEFAULT)
                if len(waits) > cap:
                    excess, keep = waits[:-cap], waits[-cap:]
                    for ci, w in enumerate(excess):
                        nop = mybir.InstNoOp(
                            name=f"{inst.name}-waitsplit{ci}",
                            sync_info=mybir.SyncInfo(on_wait=[w], on_update=[]),
                            bass_nofuse=True,
                            engine=inst.engine,
                        )
                        out.append(nop)
                    si.on_wait = keep
                out.append(inst)
            blk.instructions[:] = out


def make_nc(bc_count=BC, bw_is_zero=True, host_v=HOST_V, legalize=True):
    nc = bass.Bass()
    nt = bc_count // 128
    bf = mybir.dt.bfloat16
    f32 = mybir.dt.float32
    xh_ext = nc.declare_dram_parameter("xh", [nt * FD, 128], bf, isOutput=False)
    wct_ext = nc.declare_dram_parameter("wct", [128, F * OUT], bf,
                                        isOutput=False)
    ident_ext = nc.declare_dram_parameter("ident", [128, OUT], f32,
                                          isOutput=False)
    bcr_ext = nc.declare_dram_parameter("bc_rep", [128, OUT], f32,
                                        isOutput=False)
    kw = {}
    if host_v:
        kw["vh_ext"] = nc.declare_dram_parameter("vh", [nt * FD, 128], bf,
                                                 isOutput=False)
    else:
        kw["wbd_ext"] = nc.declare_dram_parameter("w_bd", [128, 128], bf,
                                                  isOutput=False)
        kw["bwr_ext"] = nc.declare_dram_parameter("bw_rep", [128, 1], f32,
                                                  isOutput=False)
    out_ext = nc.declare_dram_parameter("out", [bc_count, OUT], f32,
                                        isOutput=True)
    with tile.TileContext(nc) as tc:
        build_kernel(tc, out_ext, xh_ext, wct_ext, ident_ext, bcr_ext,
                     bc_count=bc_count, bw_is_zero=bw_is_zero, **kw)
    if legalize:
        legalize_waits(nc)
    return nc


def make_in_maps(fields, W, bw, Wc, bc, bc_count=BC, n_cores=N_CORES,
                 host_v=HOST_V):
    fields = np.asarray(fields, np.float32)
    w_bd, wct, ident, bc_rep, bw_rep = _host_prep_shared(W, bw, Wc, bc)
    if host_v:
        v_full = (np.einsum("fbd,de->fbe", fields.astype(BF16).astype(np.float32),
                            np.asarray(W, np.float32).astype(BF16).astype(np.float32))
                  + np.asarray(bw, np.float32)).astype(np.float32)
    in_maps = []
    for c in range(n_cores):
        sl = fields[:, c*bc_count:(c+1)*bc_count, :]
        m = {
            "xh": _host_tile_slab(sl, bc_count),
            "wct": wct, "ident": ident, "bc_rep": bc_rep,
        }
        if host_v:
            m["vh"] = _host_tile_slab(v_full[:, c*bc_count:(c+1)*bc_count, :],
                                      bc_count)
        else:
            m["w_bd"] = w_bd
            m["bw_rep"] = bw_rep
        in_maps.append(m)
    return in_maps


def run_cores(fields, W, bw, Wc, bc, bc_count=BC, n_cores=N_CORES,
              host_v=HOST_V, trace=False, tmpdir=None, ldw_opt=None):
    if ldw_opt if ldw_opt is not None else LDW_OPT:
        _apply_ldw_opt_patch()
    bw_is_zero = bool(np.all(np.asarray(bw) == 0))
    nc = make_nc(bc_count, bw_is_zero=bw_is_zero, host_v=host_v)
    in_maps = make_in_maps(fields, W, bw, Wc, bc, bc_count, n_cores, host_v)
    res = run_bass_kernel_spmd(nc, in_maps, list(range(n_cores)), trace=trace,
                               tmpdir=tmpdir)
    outs = [res.results[c]["out"] for c in range(n_cores)]
    return np.concatenate(outs, axis=0).astype(np.float32), res


def kernel(fields, W, bw, Wc, bc):
    out, _ = run_cores(fields, W, bw, Wc, bc)
    return out


# revision 28
# speedup vs baseline: 1.2669x; 1.0650x over previous
"""Trainium2 Bass kernel v2 for a bilinear field-interaction layer.

Computation (per example b):
  v[f]   = fields[f, b] @ W + bw                       # shared Dense(D)
  p[i,j] = dot(v[i], fields[j, b])  for i < j          # 780 pairs
  out[b] = p @ Wc + bc                                 # Dense(OUT)

Shapes: fields [40, 16384, 32], W [32, 32], Wc [780, 50] -> out [16384, 50].

v3 strategy (8 NeuronCores, batch-sharded 2048/core, bf16 compute):
  - Host: cast fields to bf16 and pre-tile as [nt*1280, 128] slabs so one
    dma_start_transpose (DMA xbar, ~90% of DMA BW) per b-tile yields
    x_t[32q+d, 40w+f] directly -- no on-chip StreamTranspose.  The (w, f)
    column order makes each example's 40 field-columns CONTIGUOUS, which
    is critical: strided matmul rhs streams ~4-5x slower than contiguous.
  - step1: one K=128 matmul per 512-col group with a block-diagonal
    W replicated 4x (w_bd), full-array FWL-eligible bf16 weights.
  - pairwise: per-example G[j,i] = x_j . v_i matmuls, slot-major loop so
    consecutive instructions land on 8 disjoint PE tiles (4 row-groups x
    2 col-groups) and execute concurrently; lhsT/rhs contiguous 40-col.
  - s_sb layout i-OUTERMOST: s_sb[(c2,j), i*EXc + ex] so each projection
    round's rhs is one contiguous EXc-column block (gens merged).
  - projection: 40 i-rounds accumulating Wc_i^T @ S_i into PSUM, the two
    c2 halves on diagonal quadrants running concurrently, N=EXc contig.
  - out: PE-transpose back to [b, o], +bc on evacuation, batched DMA out.
"""

import sys

sys.path.insert(0, "/opt/trn_rl_repo")

from contextlib import ExitStack

import ml_dtypes
import numpy as np

import concourse.bass as bass
import concourse.tile as tile
from concourse import mybir
from concourse._compat import with_exitstack
from concourse.bass_utils import run_bass_kernel_spmd

F, D, OUT = 40, 32, 50
FD = F * D  # 1280
NPAIR = F * (F - 1) // 2
N_CORES = 8
B_FULL = 16384
BC = B_FULL // N_CORES  # 2048 per core
CHUNK_T = 4             # b-tiles per final-stage chunk
GENS = ((0, 12), (1, 4))  # (gen index, slots per c2); 2*(12+4)=32 = b-block size
SPT = 2560              # s_sb free size per b-tile: genA 4q*480 + genB 4q*160
HOST_V = True           # if True, v is computed on host and DMA'd in
LDW_OPT = False         # if True, compile with walrus --enable-ldw-opt=true

BF16 = ml_dtypes.bfloat16


def _apply_ldw_opt_patch():
    """Rewrite the hardcoded --enable-ldw-opt=false walrus flag so LDWEIGHTS
    can be hoisted to the background weight buffer (pipelines with in-flight
    matmuls on the same PE tile)."""
    import concourse.bass_utils as _bu
    if getattr(_bu, "_ldw_patched", False):
        return
    orig = _bu.run_command

    def patched(cmd, *a, **kw):
        if isinstance(cmd, list):
            cmd = ["--enable-ldw-opt=true" if c == "--enable-ldw-opt=false"
                   else c for c in cmd]
        return orig(cmd, *a, **kw)

    _bu.run_command = patched
    _bu._ldw_patched = True


def _gen_geometry(gen):
    nslot = GENS[gen][1]
    sbase = 0 if gen == 0 else 1920      # s_sb free base within a t-slice
    gq = 40 * nslot                      # s_sb q-stride (480 / 160)
    w0 = 0 if gen == 0 else 24           # first w (example-in-block) of the gen
    return nslot, sbase, gq, w0


@with_exitstack
def build_kernel(ctx: ExitStack, tc: tile.TileContext, out_ext, xh_ext,
                 wct_ext, ident_ext, bcr_ext, wbd_ext=None, bwr_ext=None,
                 vh_ext=None, bc_count=BC, bw_is_zero=True):
    nc = tc.nc
    f32 = mybir.dt.float32
    bf16 = mybir.dt.bfloat16
    nt = bc_count // 128
    n_chunks = (nt + CHUNK_T - 1) // CHUNK_T
    host_v = vh_ext is not None

    const = ctx.enter_context(tc.tile_pool(name="const", bufs=1))
    xpool = ctx.enter_context(tc.tile_pool(name="xpool", bufs=3))
    vpool = ctx.enter_context(tc.tile_pool(name="vpool", bufs=3))
    spool = ctx.enter_context(tc.tile_pool(name="spool", bufs=2))
    apool = ctx.enter_context(tc.tile_pool(name="apool", bufs=2))
    opool = ctx.enter_context(tc.tile_pool(name="opool", bufs=2))
    gps = ctx.enter_context(tc.tile_pool(name="gps", bufs=1, space="PSUM"))
    vps = ctx.enter_context(tc.tile_pool(name="vps", bufs=2, space="PSUM"))
    accps = ctx.enter_context(tc.tile_pool(name="accps", bufs=1, space="PSUM"))
    ops = ctx.enter_context(tc.tile_pool(name="ops", bufs=1, space="PSUM"))

    # ---- constants
    wct = const.tile([128, F * OUT], bf16)
    nc.sync.dma_start(wct[:], wct_ext[:])
    ident = const.tile([128, OUT], f32)
    nc.sync.dma_start(ident[:], ident_ext[:])
    bc_rep = const.tile([128, OUT], f32)
    nc.sync.dma_start(bc_rep[:], bcr_ext[:])
    if not host_v:
        w_bd = const.tile([128, 128], bf16)
        nc.sync.dma_start(w_bd[:], wbd_ext[:])
        bw_rep = const.tile([128, 1], f32)
        nc.sync.dma_start(bw_rep[:], bwr_ext[:])

    # out viewed [t, q, w, o]
    out_r3 = out_ext.rearrange("(t q w) o -> t q w o", q=4, w=32)

    # ---- persistent pairwise PSUM banks (gap rows inited once)
    g_ps = [gps.tile([128, 480], f32, name=f"gps{q}") for q in range(4)]
    for q in range(4):
        nc.vector.memset(g_ps[q][32:64, :], 0.0)

    # evac engine assignment: big copies to ACT (352-cyc overhead amortized),
    # small ones to DVE (58-cyc overhead); keep both balanced.
    def copy_act(dst, src):
        nc.scalar.copy(dst, src)

    def copy_dve(dst, src):
        nc.vector.tensor_copy(dst, src)

    def proj_and_out(s_sb, t0, tn, exc):
        # ---- projection: acc[64c2+o, c2*exc + ex] += wct_i^T @ S_i
        acc = accps.tile([128, 2 * exc], f32, name="acc", tag="acc")
        for i in range(F):
            for c2 in range(2):
                p0 = 64 * c2
                nc.tensor.matmul(acc[p0:p0+OUT, c2*exc:(c2+1)*exc],
                                 wct[p0:p0+F, i*OUT:(i+1)*OUT],
                                 s_sb[p0:p0+F, i*exc:(i+1)*exc],
                                 start=(i == 0), stop=(i == F - 1),
                                 tile_position=(p0, p0),
                                 skip_group_check=True)

        # ---- merge parities, PE-transpose to [b, o], +bc, scatter out
        for c2 in range(2):
            p0 = 64 * c2
            a_sb = apool.tile([128, exc], f32, name=f"a_sb{c2}",
                              tag=f"a_sb{c2}")
            copy_dve(a_sb[p0:p0+OUT, :], acc[p0:p0+OUT, c2*exc:(c2+1)*exc])
            for k in range((exc + 127) // 128):
                w = min(128, exc - 128 * k)
                o_ps = ops.tile([128, OUT], f32, name="o_ps", tag="o_ps")
                nc.tensor.matmul(o_ps[0:w, :], a_sb[p0:p0+OUT, 128*k:128*k+w],
                                 ident[p0:p0+OUT, :], is_transpose=True,
                                 start=True, stop=True,
                                 tile_position=(p0, 0),
                                 skip_group_check=True)
                o_sb = opool.tile([128, OUT], f32, name="o_sb", tag="o_sb")
                nc.vector.tensor_add(o_sb[0:w, :], o_ps[0:w, :],
                                     bc_rep[0:w, :])
                # rows: (t'loc, gen-block, q, slot); DMA per (t'loc, gen)
                for tl in range(w // 64):
                    t_ = t0 + 2 * k + tl
                    for gen, nslot in GENS:
                        genb = 0 if gen == 0 else 48
                        wb = (12 * c2) if gen == 0 else (24 + 4 * c2)
                        src = o_sb[tl*64+genb : tl*64+genb+4*nslot, :]
                        dst = out_r3[t_, :, wb:wb+nslot, :]
                        nc.sync.dma_start(dst, src)

    prev = None
    for chunk in range(n_chunks):
        t0 = chunk * CHUNK_T
        tn = min(CHUNK_T, nt - t0)
        exc = tn * 64  # example-columns per c2 half of the chunk
        # s_sb[(c2,j), i*exc + ex], ex = t'*64 + genbase + q*nslot + slot
        s_sb = spool.tile([128, F * exc], bf16, name="s_sb", tag="s_sb")

        for tt in range(tn):
            t = t0 + tt
            # ---- load with DMA-xbar transpose: x_t[32q+d, 40w+f]
            x_t = xpool.tile([128, FD], bf16, name="x_t", tag="x_t")
            nc.sync.dma_start_transpose(x_t[:], xh_ext[t * FD:(t + 1) * FD, :])

            if host_v:
                v_t = vpool.tile([128, FD], bf16, name="v_t", tag="v_t")
                nc.sync.dma_start_transpose(
                    v_t[:], vh_ext[t * FD:(t + 1) * FD, :])
            else:
                # ---- step1: v^T = blockdiag(W)^T @ x^T (+bw), K=128 M=128
                v_t = vpool.tile([128, FD], bf16, name="v_t", tag="v_t")
                for s in range(3):
                    n0, n1 = s * 512, min(FD, (s + 1) * 512)
                    w = n1 - n0
                    v_ps = vps.tile([128, 512], f32, name="v_ps", tag="v_ps")
                    nc.tensor.matmul(v_ps[:, :w], w_bd[:], x_t[:, n0:n1],
                                     start=True, stop=True)
                    if bw_is_zero:
                        if s < 2:
                            copy_act(v_t[:, n0:n1], v_ps[:, :w])
                        else:
                            copy_dve(v_t[:, n0:n1], v_ps[:, :w])
                    else:
                        nc.vector.tensor_scalar_add(v_t[:, n0:n1],
                                                    v_ps[:, :w],
                                                    bw_rep[:, 0:1])

            # ---- pairwise: G[j, i] = x_j . v_i  per example
            # slot-major order: consecutive MMs hit 8 disjoint PE tiles.
            s_r = s_sb[0:104, :].rearrange("p (e x) -> p e x", e=F)
            for gen, nslot in GENS:
                _, _, gq, w0 = _gen_geometry(gen)
                for slot in range(nslot):
                    for q in range(4):
                        for c2 in range(2):
                            bp = w0 + c2 * nslot + slot
                            sl = slice(40 * bp, 40 * bp + 40)
                            nc.tensor.matmul(
                                g_ps[q][64*c2:64*c2+F, 40*slot:40*slot+F],
                                x_t[32*q:32*q+32, sl],
                                v_t[32*q:32*q+32, sl],
                                start=True, stop=True,
                                tile_position=(32*q, 64*c2))
                genb = 0 if gen == 0 else 48
                # evac: one copy per q bank into i-outer s_sb layout
                for q in range(4):
                    src = g_ps[q][0:104, 0:gq]
                    src = src.rearrange("p (s e) -> p e s", s=nslot)
                    base = tt * 64 + genb + q * nslot
                    dst = s_r[:, :, base:base+nslot]
                    if gen == 0 and q < 2:
                        copy_act(dst, src)
                    else:
                        copy_dve(dst, src)

        # software pipeline: emit the previous chunk's projection AFTER this
        # chunk's tile work so the in-order PE queue never stalls waiting for
        # the current chunk's evacuations.
        if prev is not None:
            proj_and_out(*prev)
        prev = (s_sb, t0, tn, exc)
    proj_and_out(*prev)


def _host_prep_shared(W, bw, Wc, bc):
    W = np.asarray(W, np.float32)
    w_bd = np.zeros((128, 128), np.float32)
    for q in range(4):
        w_bd[32*q:32*q+32, 32*q:32*q+32] = W
    w_bd = w_bd.astype(BF16)

    iu, ju = np.triu_indices(F, k=1)
    WcFull = np.zeros((F, F, OUT), np.float32)
    WcFull[iu, ju] = np.asarray(Wc, np.float32)
    wT = np.ascontiguousarray(np.transpose(WcFull, (1, 0, 2))).reshape(F, F*OUT)
    wct = np.zeros((128, F * OUT), np.float32)
    wct[0:40] = wT
    wct[64:104] = wT
    wct = wct.astype(BF16)

    ident = np.zeros((128, OUT), np.float32)
    for p in range(OUT):
        ident[p, p] = 1.0
        ident[64 + p, p] = 1.0
    bc_rep = np.tile(np.asarray(bc, np.float32).reshape(1, -1), (128, 1))
    bw_rep = np.tile(np.asarray(bw, np.float32).reshape(-1, 1), (4, 1))
    return w_bd, wct, ident, bc_rep, bw_rep


def _host_tile_slab(arr_f_b_d, bc_count):
    """[F, bc, D] fp32/bf16 -> [nt*1280, 128] bf16 slab for dma_start_transpose.

    Row order (w, f) within a tile so the transposed SBUF image is
    x_t[32q+d, 40w+f]: each example's 40 field-columns contiguous."""
    nt = bc_count // 128
    a = np.asarray(arr_f_b_d)
    a = a.reshape(F, nt, 4, 32, D).transpose(1, 3, 0, 2, 4)
    return np.ascontiguousarray(a.reshape(nt * FD, 128).astype(BF16))


_WAIT_CAPS = {}
_WAIT_CAP_DEFAULT = 1


def legalize_waits(nc):
    """Walrus codegen accepts only a limited number of sync-wait commands per
    instruction (1 for matmul's S3_LW path, 2 for most others).  Hoist excess
    waits onto no-op instructions on the same engine immediately before."""
    for fn in nc.m.functions:
        for blk in fn.blocks:
            out = []
            for inst in blk.instructions:
                si = getattr(inst, "sync_info", None)
                waits = list(si.on_wait) if si is not None and si.on_wait else []
                cap = _WAIT_CAPS.get(type(inst).__name__, _WAIT_CAP_DEFAULT)
                if len(waits) > cap:
                    excess, keep = waits[:-cap], waits[-cap:]
                    for ci, w in enumerate(excess):
                        nop = mybir.InstNoOp(
                            name=f"{inst.name}-waitsplit{ci}",
                            sync_info=mybir.SyncInfo(on_wait=[w], on_update=[]),
                            bass_nofuse=True,
                            engine=inst.engine,
                        )
                        out.append(nop)
                    si.on_wait = keep
                out.append(inst)
            blk.instructions[:] = out


def make_nc(bc_count=BC, bw_is_zero=True, host_v=HOST_V, legalize=True):
    nc = bass.Bass()
    nt = bc_count // 128
    bf = mybir.dt.bfloat16
    f32 = mybir.dt.float32
    xh_ext = nc.declare_dram_parameter("xh", [nt * FD, 128], bf, isOutput=False)
    wct_ext = nc.declare_dram_parameter("wct", [128, F * OUT], bf,
                                        isOutput=False)
    ident_ext = nc.declare_dram_parameter("ident", [128, OUT], f32,
                                          isOutput=False)
    bcr_ext = nc.declare_dram_parameter("bc_rep", [128, OUT], f32,
                                        isOutput=False)
    kw = {}
    if host_v:
        kw["vh_ext"] = nc.declare_dram_parameter("vh", [nt * FD, 128], bf,
                                                 isOutput=False)
    else:
        kw["wbd_ext"] = nc.declare_dram_parameter("w_bd", [128, 128], bf,
                                                  isOutput=False)
        kw["bwr_ext"] = nc.declare_dram_parameter("bw_rep", [128, 1], f32,
                                                  isOutput=False)
    out_ext = nc.declare_dram_parameter("out", [bc_count, OUT], f32,
                                        isOutput=True)
    with tile.TileContext(nc) as tc:
        build_kernel(tc, out_ext, xh_ext, wct_ext, ident_ext, bcr_ext,
                     bc_count=bc_count, bw_is_zero=bw_is_zero, **kw)
    if legalize:
        legalize_waits(nc)
    return nc


def make_in_maps(fields, W, bw, Wc, bc, bc_count=BC, n_cores=N_CORES,
                 host_v=HOST_V):
    fields = np.asarray(fields, np.float32)
    w_bd, wct, ident, bc_rep, bw_rep = _host_prep_shared(W, bw, Wc, bc)
    if host_v:
        v_full = (np.einsum("fbd,de->fbe", fields.astype(BF16).astype(np.float32),
                            np.asarray(W, np.float32).astype(BF16).astype(np.float32))
                  + np.asarray(bw, np.float32)).astype(np.float32)
    in_maps = []
    for c in range(n_cores):
        sl = fields[:, c*bc_count:(c+1)*bc_count, :]
        m = {
            "xh": _host_tile_slab(sl, bc_count),
            "wct": wct, "ident": ident, "bc_rep": bc_rep,
        }
        if host_v:
            m["vh"] = _host_tile_slab(v_full[:, c*bc_count:(c+1)*bc_count, :],
                                      bc_count)
        else:
            m["w_bd"] = w_bd
            m["bw_rep"] = bw_rep
        in_maps.append(m)
    return in_maps


def run_cores(fields, W, bw, Wc, bc, bc_count=BC, n_cores=N_CORES,
              host_v=HOST_V, trace=False, tmpdir=None, ldw_opt=None):
    if ldw_opt if ldw_opt is not None else LDW_OPT:
        _apply_ldw_opt_patch()
    bw_is_zero = bool(np.all(np.asarray(bw) == 0))
    nc = make_nc(bc_count, bw_is_zero=bw_is_zero, host_v=host_v)
    in_maps = make_in_maps(fields, W, bw, Wc, bc, bc_count, n_cores, host_v)
    res = run_bass_kernel_spmd(nc, in_maps, list(range(n_cores)), trace=trace,
                               tmpdir=tmpdir)
    outs = [res.results[c]["out"] for c in range(n_cores)]
    return np.concatenate(outs, axis=0).astype(np.float32), res


def kernel(fields, W, bw, Wc, bc):
    out, _ = run_cores(fields, W, bw, Wc, bc)
    return out
